# revision 11
# baseline (speedup 1.0000x reference)
"""Trainium2 Bass kernel for nn_CloudSecurityGNN (3-layer GAT + MLP heads).

Sharding: nodes split into 8 contiguous shards (6250/core, padded to 6400 =
50 groups of 128); edges sharded by dst core, sorted by dst, grouped per 128
dst nodes, padded to a uniform subtile count S_MAX (subtile = 128 edges).

Per layer:
  dense (replicated on every core): full table H'[51200, 272] bf16 rows =
    [h(256) | als(4 f32 packed as 8 bf16)] plus ALD[51200, 4] f32, computed
    as one matmul per 128-node tile with rhs [K, 264] = [W | W@As | W@Ad].
  edge (sharded by dst): per group, batched indirect-DMA gather of H' rows by
    src and ALD rows by dst; p = exp(leaky_relu(als+ald)) (logits are small,
    max-subtraction provably unnecessary for this data); selector-matrix
    matmul S^T @ [p*h | 4p] accumulates numerators + denominators in PSUM;
    finalize divides, head-averages, applies relu (layers 1-2).
  emb chunks (2/layer) are allgathered in bf16 between layers so the dense
  phase can be replicated instead of allgathering the 28 MB table.
Layer 3 adds on-device MLP heads (anomaly/risk/resource) and graph mean-pool
partial sums (AllReduce); the tiny [64 x *] graph MLP finishes on host.
"""

import math

import numpy as np

import concourse.bacc as bacc
import concourse.bass as bass
import concourse.mybir as mybir
import concourse.tile as tile
from concourse.masks import make_identity

P = 128
FP32 = mybir.dt.float32
BF16 = mybir.dt.bfloat16
I32 = mybir.dt.int32
AF = mybir.ActivationFunctionType
ALU = mybir.AluOpType

N_CORES = 8
NEG_SLOPE = 0.2
DENSE_BATCH = 8


# ---------------------------------------------------------------------------
# Host-side preprocessing
# ---------------------------------------------------------------------------

def _plan(N, G, H, C, F_IN):
    nloc_real = N // N_CORES
    n_chunks = 2
    groups = math.ceil(nloc_real / P)
    groups = math.ceil(groups / n_chunks) * n_chunks
    nloc = groups * P
    return dict(N=N, G=G, H=H, C=C, F_IN=F_IN, HC=H * C,
                NLOC_REAL=nloc_real, GROUPS=groups, N_CHUNKS=n_chunks,
                NLOC=nloc, CHUNK_LOC=nloc // n_chunks, NPHYS=nloc * N_CORES)


def _phys_of_global(pl):
    n = np.arange(pl['N'], dtype=np.int64)
    k = n // pl['NLOC_REAL']
    l = n - k * pl['NLOC_REAL']
    c = l // pl['CHUNK_LOC']
    return (c * (pl['CHUNK_LOC'] * N_CORES) + k * pl['CHUNK_LOC']
            + (l % pl['CHUNK_LOC'])).astype(np.int32)


def _edge_schedule(pl, edge_index):
    """Returns per-core [128, GROUPS*S_MAX] arrays: src phys idx (gather),
    dst phys idx (ALD gather), dst_rel bf16 (0..127 in-group, -1 = pad)."""
    import ml_dtypes
    N = pl['N']
    loop = np.arange(N, dtype=np.int64)
    src = np.concatenate([edge_index[0].astype(np.int64), loop])
    dst = np.concatenate([edge_index[1].astype(np.int64), loop])
    phys = _phys_of_global(pl)

    kd = dst // pl['NLOC_REAL']
    ld = dst - kd * pl['NLOC_REAL']
    grp, rel = ld // P, ld % P

    order = np.argsort(kd * pl['GROUPS'] + grp, kind='stable')
    src, dst, kd, grp, rel = (a[order] for a in (src, dst, kd, grp, rel))

    cg = kd * pl['GROUPS'] + grp
    counts = np.bincount(cg, minlength=N_CORES * pl['GROUPS'])
    s_max = int(math.ceil(counts.max() / P))
    cap = s_max * P

    src_a = np.zeros((N_CORES, pl['GROUPS'], cap), np.int32)
    dst_a = np.zeros((N_CORES, pl['GROUPS'], cap), np.int32)
    rel_a = np.full((N_CORES, pl['GROUPS'], cap), -1.0, np.float32)

    starts = np.zeros(N_CORES * pl['GROUPS'] + 1, np.int64)
    np.cumsum(counts, out=starts[1:])
    pos = cg * cap + (np.arange(len(src)) - starts[cg])
    src_a.reshape(-1)[pos] = phys[src]
    dst_a.reshape(-1)[pos] = phys[dst]
    rel_a.reshape(-1)[pos] = rel

    def part_major(a):  # [core, groups, s_max, P] -> [core, P, groups*s_max]
        a = a.reshape(N_CORES, pl['GROUPS'] * s_max, P)
        return np.ascontiguousarray(np.transpose(a, (0, 2, 1)))

    return (part_major(src_a), part_major(dst_a),
            part_major(rel_a).astype(ml_dtypes.bfloat16), s_max)


def _host_constants(pl, inp):
    import ml_dtypes
    H, C, HC = pl['H'], pl['C'], pl['HC']

    def wprime(W, a_s, a_d):
        Wr = W.reshape(W.shape[0], H, C)
        WAs = np.einsum('fhc,hc->fh', Wr, a_s)
        WAd = np.einsum('fhc,hc->fh', Wr, a_d)
        return np.concatenate([W, WAs, WAd], axis=1).astype(np.float32)

    Wp = [wprime(np.asarray(inp['W1']), np.asarray(inp['as1']), np.asarray(inp['ad1'])),
          wprime(np.asarray(inp['W2']), np.asarray(inp['as2']), np.asarray(inp['ad2'])),
          wprime(np.asarray(inp['W3']), np.asarray(inp['as3']), np.asarray(inp['ad3']))]

    b1, b2, b3 = (np.asarray(inp[k], np.float32) for k in ('b1', 'b2', 'b3'))
    bias_rows = [None,
                 b1 @ Wp[1] if np.any(b1) else None,
                 b2 @ Wp[2] if np.any(b2) else None]

    phys = _phys_of_global(pl)
    xT = np.zeros((pl['F_IN'], pl['NPHYS']), np.float32)
    xT[:, phys] = np.asarray(inp['x']).T
    xT = xT.astype(ml_dtypes.bfloat16)

    Wh1 = np.concatenate([inp['aw1'], inp['rw1'], inp['cw1']], axis=1).astype(np.float32)
    hb1 = np.concatenate([inp['ab1'], inp['rb1'], inp['cb1']]).astype(np.float32)[:, None]
    Wh2 = np.zeros((97, 7), np.float32)
    Wh2[0:32, 0] = np.asarray(inp['aw2'])[:, 0]
    Wh2[32:64, 1] = np.asarray(inp['rw2'])[:, 0]
    Wh2[64:96, 2:7] = np.asarray(inp['cw2'])
    Wh2[96, 0] = np.asarray(inp['ab2'])[0]
    Wh2[96, 1] = np.asarray(inp['rb2'])[0]
    Wh2[96, 2:7] = np.asarray(inp['cb2'])

    batch = np.asarray(inp['batch'])
    G = pl['G']
    Sb = np.zeros((N_CORES, P, pl['GROUPS'] * G), np.float32)
    l_all = np.arange(pl['NLOC_REAL'])
    for k in range(N_CORES):
        b_k = batch[k * pl['NLOC_REAL']:(k + 1) * pl['NLOC_REAL']]
        Sb[k, l_all % P, (l_all // P) * G + b_k] = 1.0

    iota = np.ascontiguousarray(
        np.broadcast_to(np.arange(P, dtype=np.float32), (P, P))
    ).astype(ml_dtypes.bfloat16)

    return dict(Wp=Wp, bias_rows=bias_rows, xT=xT, Wh1=Wh1, hb1=hb1, Wh2=Wh2,
                Sb=Sb, iota=iota, b3=b3 if np.any(b3) else None)


# ---------------------------------------------------------------------------
# Device program
# ---------------------------------------------------------------------------

def build_program(pl, s_max, has_b12, has_b3):
    nc = bacc.Bacc("TRN2", target_bir_lowering=False, debug=False,
                   num_devices=N_CORES)
    GROUPS, NPHYS, NLOC, G = pl['GROUPS'], pl['NPHYS'], pl['NLOC'], pl['G']
    HC, H, C, F_IN = pl['HC'], pl['H'], pl['C'], pl['F_IN']
    N_CHUNKS, CHUNK_LOC = pl['N_CHUNKS'], pl['CHUNK_LOC']
    CHUNK_PHYS = CHUNK_LOC * N_CORES
    GR_PER_CH = GROUPS // N_CHUNKS
    SM = s_max
    TW = HC + 2 * H      # 264: dense matmul out  [h | als | ald]
    TROW = HC + 2 * H    # 264 bf16 = 528 B table row [h | als(f32x4)]
    RW = HC + H          # 260: edge-matmul rhs  [p*h | 4p]
    replica = [list(range(N_CORES))]

    n_tiles = NPHYS // P
    tiles_per_chunk = CHUNK_PHYS // P
    assert n_tiles % DENSE_BATCH == 0 and tiles_per_chunk % DENSE_BATCH == 0

    # ---- I/O ----
    xT_t = nc.dram_tensor("xT", [F_IN, NPHYS], BF16, kind="ExternalInput")
    srci_t = nc.dram_tensor("src_idx", [P, GROUPS * SM], I32, kind="ExternalInput")
    dsti_t = nc.dram_tensor("dst_idx", [P, GROUPS * SM], I32, kind="ExternalInput")
    drel_t = nc.dram_tensor("dst_rel", [P, GROUPS * SM], BF16, kind="ExternalInput")
    iota_t = nc.dram_tensor("iota", [P, P], BF16, kind="ExternalInput")
    sb_t = nc.dram_tensor("Sb", [P, GROUPS * G], FP32, kind="ExternalInput")
    wp_t = [nc.dram_tensor(f"Wp{i+1}", [F_IN if i == 0 else C, TW], BF16,
                           kind="ExternalInput") for i in range(3)]
    wh1_t = nc.dram_tensor("Wh1", [C, 96], FP32, kind="ExternalInput")
    hb1_t = nc.dram_tensor("hb1", [96, 1], FP32, kind="ExternalInput")
    wh2_t = nc.dram_tensor("Wh2", [97, 7], FP32, kind="ExternalInput")
    brow_t = [None] + [nc.dram_tensor(f"brow{i+1}", [1, TW], FP32,
                                      kind="ExternalInput") if has_b12 else None
                       for i in (1, 2)]
    b3_t = (nc.dram_tensor("b3rep", [1, C], FP32, kind="ExternalInput")
            if has_b3 else None)

    emb_out = nc.dram_tensor("emb_out", [NLOC, C], FP32, kind="ExternalOutput")
    arr_out = nc.dram_tensor("arr_out", [NLOC, 7], FP32, kind="ExternalOutput")
    pooled_out = nc.dram_tensor("pooled_out", [G, C], FP32, kind="ExternalOutput")

    # ---- internal DRAM ----
    tables = [nc.dram_tensor(f"tab{i}", [NPHYS, TROW], BF16, kind="Internal")
              for i in range(3)]
    alds = [nc.dram_tensor(f"ald{i}", [NPHYS, H], FP32, kind="Internal")
            for i in range(3)]
    # emb is stored TRANSPOSED [C, nodes]; AllGather (partition concat) stacks
    # rank k's channels at rows [C*k, C*(k+1)) of full_emb.
    loc_emb = [[nc.dram_tensor(f"locemb{i}_{c}", [C, CHUNK_LOC], BF16, kind="Internal")
                for c in range(N_CHUNKS)] for i in range(2)]
    full_emb = [[nc.dram_tensor(f"fullemb{i}_{c}", [C * N_CORES, CHUNK_LOC], BF16,
                                kind="Internal", addr_space="Shared")
                 for c in range(N_CHUNKS)] for i in range(2)]
    pool_loc = nc.dram_tensor("pool_loc", [G, C], FP32, kind="Internal")
    pool_red = nc.dram_tensor("pool_red", [G, C], FP32, kind="Internal",
                              addr_space="Shared")

    with tile.TileContext(nc) as tc:
        with tc.tile_pool(name="const", bufs=1) as cpool, \
             tc.tile_pool(name="sbuf", bufs=3) as spool, \
             tc.tile_pool(name="edge", bufs=2) as epool, \
             tc.tile_pool(name="psD", bufs=2, space="PSUM") as psD, \
             tc.tile_pool(name="psG", bufs=2, space="PSUM") as psG, \
             tc.tile_pool(name="psS", bufs=3, space="PSUM") as psS, \
             tc.tile_pool(name="psP", bufs=1, space="PSUM") as psP:

            # ---- persistent SBUF constants ----
            srci = cpool.tile([P, GROUPS * SM], I32)
            dsti = cpool.tile([P, GROUPS * SM], I32)
            drel = cpool.tile([P, GROUPS * SM], BF16)
            iota = cpool.tile([P, P], BF16)
            sb_c = cpool.tile([P, GROUPS * G], FP32)
            wp_c = [cpool.tile([F_IN if i == 0 else C, TW], BF16, name=f"wpc{i}")
                    for i in range(3)]
            wh1_c = cpool.tile([C, 96], FP32)
            hb1_c = cpool.tile([96, 1], FP32)
            wh2_c = cpool.tile([97, 7], FP32)
            ident = cpool.tile([P, P], FP32)
            for dst_tile, src_t in ((srci, srci_t), (dsti, dsti_t), (drel, drel_t),
                                    (iota, iota_t), (sb_c, sb_t), (wh1_c, wh1_t),
                                    (hb1_c, hb1_t), (wh2_c, wh2_t)):
                nc.sync.dma_start(out=dst_tile[:], in_=src_t.ap())
            for i in range(3):
                nc.sync.dma_start(out=wp_c[i][:], in_=wp_t[i].ap())
            make_identity(nc, ident[:])
            brow_c = [None, None, None]
            for i in (1, 2):
                if brow_t[i] is not None:
                    brow_c[i] = cpool.tile([1, TW], FP32, name=f"browc{i}")
                    nc.sync.dma_start(out=brow_c[i][:], in_=brow_t[i].ap())
            b3_c = None
            if b3_t is not None:
                b3_c = cpool.tile([1, C], FP32)
                nc.sync.dma_start(out=b3_c[:], in_=b3_t.ap())

            def emit_dense(L, lhs3, db, t0):
                """db matmul tiles + evicts + batched table/ALD stores."""
                stag = spool.tile([P, db * TROW], BF16, tag="stag", name="stag")
                stag3 = stag[:].rearrange("p (j q) -> p j q", j=db)
                ald_s = spool.tile([P, db * H], FP32, tag="aldstag", name="ald_s")
                ald3 = ald_s[:].rearrange("p (j q) -> p j q", j=db)
                for j in range(db):
                    ps = psD.tile([P, TW], FP32, tag="psD", name="psd")
                    nc.tensor.matmul(out=ps[:], lhsT=lhs3[:, j, :],
                                     rhs=wp_c[L][:], start=True, stop=True)
                    h_dst = stag3[:, j, 0:HC]
                    als_dst = stag3[:, j, HC:HC + 2 * H].bitcast(FP32)
                    if brow_c[L] is not None:
                        nc.vector.tensor_tensor(
                            out=h_dst, in0=ps[:, 0:HC],
                            in1=brow_c[L][:, 0:HC].partition_broadcast(P),
                            op=ALU.add)
                        nc.vector.tensor_tensor(
                            out=als_dst, in0=ps[:, HC:HC + H],
                            in1=brow_c[L][:, HC:HC + H].partition_broadcast(P),
                            op=ALU.add)
                        nc.vector.tensor_tensor(
                            out=ald3[:, j, :], in0=ps[:, HC + H:HC + 2 * H],
                            in1=brow_c[L][:, HC + H:HC + 2 * H].partition_broadcast(P),
                            op=ALU.add)
                    elif j % 2 == 0:
                        nc.vector.tensor_copy(out=h_dst, in_=ps[:, 0:HC])
                        nc.scalar.activation(out=als_dst, in_=ps[:, HC:HC + H],
                                             func=AF.Copy)
                        nc.scalar.activation(out=ald3[:, j, :],
                                             in_=ps[:, HC + H:HC + 2 * H],
                                             func=AF.Copy)
                    else:
                        nc.scalar.activation(out=h_dst, in_=ps[:, 0:HC],
                                             func=AF.Copy)
                        nc.vector.tensor_copy(out=als_dst, in_=ps[:, HC:HC + H])
                        nc.vector.tensor_copy(out=ald3[:, j, :],
                                              in_=ps[:, HC + H:HC + 2 * H])
                nc.sync.dma_start(
                    out=tables[L].ap()[t0 * P:(t0 + db) * P, :]
                        .rearrange("(j p) c -> p j c", p=P),
                    in_=stag3)
                nc.scalar.dma_start(
                    out=alds[L].ap()[t0 * P:(t0 + db) * P, :]
                        .rearrange("(j p) c -> p j c", p=P),
                    in_=ald3)

            TS = CHUNK_LOC // P
            db2 = next(d for d in range(min(DENSE_BATCH, TS), 0, -1) if TS % d == 0)

            for L in range(3):
                # ================= dense phase =================
                if L == 0:
                    for m in range(n_tiles // DENSE_BATCH):
                        t0 = m * DENSE_BATCH
                        lhs = spool.tile([F_IN, DENSE_BATCH * P], BF16, tag="lhs")
                        nc.sync.dma_start(
                            out=lhs[:], in_=xT_t.ap()[:, t0 * P:(t0 + DENSE_BATCH) * P])
                        emit_dense(L, lhs[:].rearrange("p (j q) -> p j q",
                                                       j=DENSE_BATCH), DENSE_BATCH, t0)
                else:
                    for ch in range(N_CHUNKS):
                        for k in range(N_CORES):
                            stripe = spool.tile([C, CHUNK_LOC], BF16, tag="stripe")
                            nc.sync.dma_start(
                                out=stripe[:],
                                in_=full_emb[L - 1][ch].ap()[C * k:C * (k + 1), :])
                            st3 = stripe[:].rearrange("p (j q) -> p j q", q=P)
                            for mb in range(TS // db2):
                                t0 = (ch * N_CORES + k) * TS + mb * db2
                                emit_dense(L, st3[:, mb * db2:(mb + 1) * db2, :],
                                           db2, t0)

                # ================= edge phase =================
                if L == 2:
                    pool_ps = psP.tile([G, C], FP32, tag="pool")
                for g in range(GROUPS):
                    gs = slice(g * SM, (g + 1) * SM)
                    gat = epool.tile([P, SM * TROW], BF16, tag="gat")
                    nc.gpsimd.indirect_dma_start(
                        out=gat[:], out_offset=None, in_=tables[L].ap(),
                        in_offset=bass.IndirectOffsetOnAxis(ap=srci[:, gs], axis=0))
                    aldg = epool.tile([P, SM * H], FP32, tag="aldg")
                    nc.gpsimd.indirect_dma_start(
                        out=aldg[:], out_offset=None, in_=alds[L].ap(),
                        in_offset=bass.IndirectOffsetOnAxis(ap=dsti[:, gs], axis=0))
                    gat3 = gat[:].rearrange("p (t c) -> p t c", t=SM)
                    als_v = gat3[:, :, HC:HC + 2 * H].bitcast(FP32)   # [P,SM,H] f32
                    elog = epool.tile([P, SM * H], FP32, tag="elog")
                    nc.vector.tensor_tensor(
                        out=elog[:].rearrange("p (t h) -> p t h", t=SM),
                        in0=als_v,
                        in1=aldg[:].rearrange("p (t h) -> p t h", t=SM), op=ALU.add)
                    esc = epool.tile([P, SM * H], FP32, tag="esc")
                    nc.vector.tensor_scalar_mul(esc[:], elog[:], NEG_SLOPE)
                    nc.vector.tensor_tensor(out=elog[:], in0=elog[:], in1=esc[:],
                                            op=ALU.max)
                    p_bf = epool.tile([P, SM * H], BF16, tag="pbf")
                    nc.scalar.activation(out=p_bf[:], in_=elog[:], func=AF.Exp)
                    p3 = p_bf[:].rearrange("p (t h) -> p t h", t=SM)
                    smat = epool.tile([P, SM * P], BF16, tag="smat")
                    smat3 = smat[:].rearrange("p (t q) -> p t q", t=SM)
                    nc.vector.tensor_tensor(
                        out=smat3,
                        in0=drel[:, gs].unsqueeze(2).to_broadcast([P, SM, P]),
                        in1=iota[:].unsqueeze(1).to_broadcast([P, SM, P]),
                        op=ALU.is_equal)
                    rhs = epool.tile([P, SM * RW], BF16, tag="rhs")
                    rhs3 = rhs[:].rearrange("p (t c) -> p t c", t=SM)
                    for h in range(H):
                        nc.vector.tensor_tensor(
                            out=rhs3[:, :, h * C:(h + 1) * C],
                            in0=gat3[:, :, h * C:(h + 1) * C],
                            in1=p3[:, :, h:h + 1].to_broadcast([P, SM, C]),
                            op=ALU.mult)
                    nc.scalar.activation(out=rhs3[:, :, HC:HC + H], in_=p3,
                                         func=AF.Copy, scale=4.0)
                    ps = psG.tile([P, RW], FP32, tag="psG")
                    for t in range(SM):
                        nc.tensor.matmul(out=ps[:], lhsT=smat3[:, t, :],
                                         rhs=rhs3[:, t, :],
                                         start=(t == 0), stop=(t == SM - 1))
                    den = spool.tile([P, H], FP32, tag="den")
                    nc.vector.tensor_scalar_max(den[:], ps[:, HC:HC + H], 1e-30)
                    rec = spool.tile([P, H], FP32, tag="rec")
                    nc.vector.reciprocal(out=rec[:], in_=den[:])
                    ta = spool.tile([P, HC], FP32, tag="ta")
                    nc.vector.tensor_tensor(
                        out=ta[:].rearrange("p (h c) -> p h c", h=H),
                        in0=ps[:, 0:HC].rearrange("p (h c) -> p h c", h=H),
                        in1=rec[:].unsqueeze(2).to_broadcast([P, H, C]),
                        op=ALU.mult)
                    s1 = spool.tile([P, 2 * C], FP32, tag="s1")
                    nc.vector.tensor_tensor(out=s1[:], in0=ta[:, 0:2 * C],
                                            in1=ta[:, 2 * C:HC], op=ALU.add)
                    embf = spool.tile([P, C], FP32, tag="embf")
                    nc.vector.tensor_tensor(out=embf[:], in0=s1[:, 0:C],
                                            in1=s1[:, C:2 * C], op=ALU.add)
                    if L == 2 and b3_c is not None:
                        nc.vector.tensor_tensor(
                            out=embf[:], in0=embf[:],
                            in1=b3_c[:].partition_broadcast(P), op=ALU.add)
                    embT_ps = psS.tile([C, P], FP32, tag="psS")
                    nc.tensor.transpose(out=embT_ps[:], in_=embf[:],
                                        identity=ident[:])
                    if L < 2:
                        embTb = spool.tile([C, P], BF16, tag="embTb")
                        nc.scalar.activation(out=embTb[:], in_=embT_ps[:],
                                             func=AF.Relu)
                        ch = g // GR_PER_CH
                        r0 = (g - ch * GR_PER_CH) * P
                        nc.sync.dma_start(out=loc_emb[L][ch].ap()[:, r0:r0 + P],
                                          in_=embTb[:])
                        if g % GR_PER_CH == GR_PER_CH - 1:
                            nc.gpsimd.collective_compute(
                                "AllGather", ALU.bypass, replica_groups=replica,
                                ins=[loc_emb[L][ch].ap()],
                                outs=[full_emb[L][ch].ap()])
                    else:
                        nc.sync.dma_start(out=emb_out.ap()[g * P:(g + 1) * P, :],
                                          in_=embf[:])
                        nc.tensor.matmul(out=pool_ps[:],
                                         lhsT=sb_c[:, g * G:(g + 1) * G],
                                         rhs=embf[:], start=(g == 0),
                                         stop=(g == GROUPS - 1),
                                         skip_group_check=True)
                        embT = spool.tile([C, P], FP32, tag="embT")
                        nc.vector.tensor_copy(out=embT[:], in_=embT_ps[:])
                        h1_ps = psS.tile([96, P], FP32, tag="psS")
                        nc.tensor.matmul(out=h1_ps[:], lhsT=wh1_c[:], rhs=embT[:],
                                         start=True, stop=True)
                        r1 = spool.tile([97, P], FP32, tag="r1")
                        nc.scalar.activation(out=r1[0:96, :], in_=h1_ps[:],
                                             func=AF.Relu, bias=hb1_c[:])
                        nc.gpsimd.memset(r1[96:97, :], 1.0)
                        h2_ps = psS.tile([P, 7], FP32, tag="psS")
                        nc.tensor.matmul(out=h2_ps[:], lhsT=r1[:], rhs=wh2_c[:],
                                         start=True, stop=True)
                        arrs = spool.tile([P, 7], FP32, tag="arrs")
                        nc.scalar.activation(out=arrs[:, 0:2], in_=h2_ps[:, 0:2],
                                             func=AF.Sigmoid)
                        nc.vector.tensor_copy(out=arrs[:, 2:7], in_=h2_ps[:, 2:7])
                        nc.scalar.dma_start(out=arr_out.ap()[g * P:(g + 1) * P, :],
                                            in_=arrs[:])
                if L == 2:
                    pool_s = spool.tile([G, C], FP32, tag="pool_s")
                    nc.vector.tensor_copy(out=pool_s[:], in_=pool_ps[:])
                    nc.sync.dma_start(out=pool_loc.ap(), in_=pool_s[:])
                    nc.gpsimd.collective_compute(
                        "AllReduce", ALU.add, replica_groups=replica,
                        ins=[pool_loc.ap()], outs=[pool_red.ap()])
                    nc.sync.dma_start(out=pooled_out.ap(), in_=pool_red.ap())

    nc.compile()
    return nc


# ---------------------------------------------------------------------------
# Entry point
# ---------------------------------------------------------------------------

def make_in_maps(pl, inp):
    import ml_dtypes
    src_a, dst_a, rel_a, s_max = _edge_schedule(pl, np.asarray(inp['edge_index']))
    hc = _host_constants(pl, inp)
    has_b12 = any(hc['bias_rows'][i] is not None for i in (1, 2))
    in_maps = []
    for k in range(N_CORES):
        m = dict(xT=np.asarray(hc['xT']),
                 src_idx=src_a[k], dst_idx=dst_a[k], dst_rel=rel_a[k],
                 iota=np.asarray(hc['iota']), Sb=hc['Sb'][k],
                 Wh1=hc['Wh1'], hb1=hc['hb1'], Wh2=hc['Wh2'])
        for i in range(3):
            m[f"Wp{i+1}"] = hc['Wp'][i].astype(ml_dtypes.bfloat16)
        if has_b12:
            for i in (1, 2):
                row = hc['bias_rows'][i]
                if row is None:
                    row = np.zeros(pl['HC'] + 2 * pl['H'], np.float32)
                m[f"brow{i+1}"] = row.astype(np.float32)[None, :]
        if hc['b3'] is not None:
            m["b3rep"] = hc['b3'][None, :]
        in_maps.append(m)
    return in_maps, s_max, has_b12, hc['b3'] is not None


def assemble_outputs(pl, inp, results):
    nreal = pl['NLOC_REAL']
    emb = np.concatenate([results[k]['emb_out'][:nreal] for k in range(N_CORES)])
    arr = np.concatenate([results[k]['arr_out'][:nreal] for k in range(N_CORES)])
    sums = results[0]['pooled_out']
    batch = np.asarray(inp['batch'])
    cnt = np.bincount(batch, minlength=pl['G']).astype(np.float32)
    pooled = sums / np.maximum(cnt, 1.0)[:, None]
    gw1, gb1 = np.asarray(inp['gw1']), np.asarray(inp['gb1'])
    gw2, gb2 = np.asarray(inp['gw2']), np.asarray(inp['gb2'])
    gl = np.maximum(pooled @ gw1 + gb1, 0.0) @ gw2 + gb2
    return (np.ascontiguousarray(emb, np.float32),
            np.ascontiguousarray(arr[:, 0:1], np.float32),
            np.ascontiguousarray(arr[:, 1:2], np.float32),
            np.ascontiguousarray(arr[:, 2:7], np.float32),
            np.ascontiguousarray(gl, np.float32))


def kernel(**inputs):
    N, F_IN = np.asarray(inputs['x']).shape
    H, C = np.asarray(inputs['as1']).shape
    pl = _plan(N, 64, H, C, F_IN)
    in_maps, s_max, has_b12, has_b3 = make_in_maps(pl, inputs)
    nc = build_program(pl, s_max, has_b12, has_b3)
    from concourse.bass_utils import run_bass_kernel_spmd
    res = run_bass_kernel_spmd(nc, in_maps, core_ids=list(range(N_CORES)))
    return assemble_outputs(pl, inputs, res.results)
